# revision 1
# baseline (speedup 1.0000x reference)
"""Chamfer loss on 8 Trainium2 NeuronCores.

Data-parallel over batch B=8: core c handles batch element c and computes
sum_n sqrt(min_m d[n,m]) and sum_m sqrt(min_n d[n,m]) for its element;
the host combines the 16 partial sums into the final scalar mean.

Device algorithm (per core, per direction):
  d[n,m] = ||a_n||^2 + ||b_m||^2 - 2 a.b is produced as ONE K=24 bf16
  matmul per (128-row, 512-col) tile: each fp32 coordinate is split into
  3 bf16 components (hi/mid/lo) and the 6 dominant cross products are
  assigned to matmul rows, plus 3 rows for each squared-norm (split to
  bf16 triples against a row of ones). This keeps the TensorE at its full
  1 column/cycle rate (native fp32 matmul is 4x slower) while keeping
  ~1e-7 absolute accuracy in the distances.

  Row minima: the PE writes distance tiles to PSUM; ScalarE copies every
  other 1024-wide chunk to SBUF; VectorE then consumes chunk PAIRS with
  tensor_tensor_scan(op0=min, op1=min) - one PSUM chunk + one SBUF chunk
  per instruction, i.e. 2 distance values per cycle per lane, with the
  running row-min carried through the scan's initial value. The scan
  output is a stride-0 broadcast AP so the final state lands in a [128,1]
  cell. relu + sqrt (+ free-dim accumulation) run on ScalarE/VectorE;
  the 128-lane partial sums are shipped to the host (2x128 floats/core).
"""

import numpy as np
import ml_dtypes

import concourse.bass as bass
import concourse.mybir as mybir
import concourse.tile as tile
from concourse import bacc
from concourse.bass_utils import run_bass_kernel_spmd

B = 8
N = 8192          # points per set (a and b identical here)
K = 24            # augmented contraction rows
NT = N // 128     # 64 n-tiles of 128 query points
NQ = 4            # m-quads of 2048 (= one PSUM chunk + one SBUF chunk)
F32 = mybir.dt.float32
BF16 = mybir.dt.bfloat16
BF = ml_dtypes.bfloat16

_NC_CACHE = None


def _split3(v32: np.ndarray):
    """fp32 -> (hi, mid, lo) bf16 triple with hi+mid+lo == v to ~2^-24 rel."""
    v1 = v32.astype(BF)
    r = v32 - v1.astype(np.float32)
    v2 = r.astype(BF)
    v3 = (r - v2.astype(np.float32)).astype(BF)
    return v1, v2, v3


def _operands(pts: np.ndarray):
    """pts [N,3] fp32 -> (w [24,N] bf16 weight-side, m [24,N] bf16 moving-side).

    Row pairing (per coordinate k, g = split3(-2*coord), h = split3(coord)):
      w rows: g1 g1 g2 g2 g1 g3     m rows: h1 h2 h1 h2 h3 h1
    so sum_r w[r]*m[r] = -2*coord_a*coord_b up to ~2^-26 terms.
    Rows 18-20: w = split3(||a||^2), m = 1.  Rows 21-23: w = 1, m = split3(||b||^2).
    """
    s = (pts.astype(np.float64) ** 2).sum(axis=1).astype(np.float32)
    s1, s2, s3 = _split3(s)
    w = np.empty((K, pts.shape[0]), dtype=BF)
    m = np.empty((K, pts.shape[0]), dtype=BF)
    for k in range(3):
        c = pts[:, k].astype(np.float32)
        g1, g2, g3 = _split3(-2.0 * c)
        h1, h2, h3 = _split3(c)
        r = 6 * k
        w[r + 0], w[r + 1], w[r + 2] = g1, g1, g2
        w[r + 3], w[r + 4], w[r + 5] = g2, g1, g3
        m[r + 0], m[r + 1], m[r + 2] = h1, h2, h1
        m[r + 3], m[r + 4], m[r + 5] = h2, h3, h1
    one = np.ones(pts.shape[0], dtype=BF)
    w[18], w[19], w[20] = s1, s2, s3
    m[18], m[19], m[20] = one, one, one
    w[21], w[22], w[23] = one, one, one
    m[21], m[22], m[23] = s1, s2, s3
    return w, m


def _build_nc():
    nc = bacc.Bacc(None)
    wa_d = nc.declare_dram_parameter("wa", [K, N], BF16, isOutput=False)
    mb_d = nc.declare_dram_parameter("mb", [K, N], BF16, isOutput=False)
    wb_d = nc.declare_dram_parameter("wb", [K, N], BF16, isOutput=False)
    ma_d = nc.declare_dram_parameter("ma", [K, N], BF16, isOutput=False)
    out_d = nc.declare_dram_parameter("out", [2, 128], F32, isOutput=True)

    F16 = mybir.dt.float16
    MIN = mybir.AluOpType.min

    with tile.TileContext(nc) as tc:
        with (
            tc.tile_pool(name="const", bufs=1) as cpool,
            tc.tile_pool(name="psum", bufs=2, space="PSUM") as pspool,
            tc.tile_pool(name="scopy", bufs=3) as sbpool,
            tc.tile_pool(name="tmin", bufs=2) as tpool,
            tc.tile_pool(name="strip", bufs=2) as stpool,
        ):
            # operands replicated at partition offsets 0/32/64/96 so four
            # matmuls can run concurrently in distinct 32-row PE groups
            wa_t = cpool.tile([128, N], BF16, tag="wa")
            mb_t = cpool.tile([128, N], BF16, tag="mb")
            wb_t = cpool.tile([128, N], BF16, tag="wb")
            ma_t = cpool.tile([128, N], BF16, tag="ma")
            for t, dram in ((wa_t, wa_d), (mb_t, mb_d), (wb_t, wb_d), (ma_t, ma_d)):
                for g in range(4):
                    nc.sync.dma_start(out=t[32 * g:32 * g + K, :], in_=dram[:])

            def emit_chunk(ck, w_t, m_t, nt, q):
                for g in range(4):
                    o = q * 2048 + g * 512
                    nc.tensor.matmul(
                        out=ck[:, g * 512:(g + 1) * 512],
                        lhsT=w_t[32 * g:32 * g + K, nt * 128:(nt + 1) * 128],
                        rhs=m_t[32 * g:32 * g + K, o:o + 512],
                        start=True, stop=True,
                        tile_position=(32 * g, 0))

            for p, (w_t, m_t) in enumerate(((wa_t, mb_t), (wb_t, ma_t))):
                strip_a = stpool.tile([128, NT], F32, tag="stripa")
                strip_b = stpool.tile([128, NT], F32, tag="stripb")
                for nt in range(NT):
                    # m in 4 chunks of 2048. Chunk 3 is min-reduced by the
                    # VectorE straight out of PSUM (early, so its slot
                    # frees fast); chunks 0-2 go via ScalarE to fp16 SBUF,
                    # then a 2x-mode TT-min tree + folded reduce. No PSUM
                    # tile outlives its chunk turn -> 2-slot rotation
                    # pipelines cleanly.
                    c3 = pspool.tile([128, 2048], F32, tag="ps")
                    emit_chunk(c3, w_t, m_t, nt, 3)
                    nc.vector.tensor_reduce(out=strip_a[:, nt:nt + 1],
                                            in_=c3[:],
                                            axis=mybir.AxisListType.X, op=MIN)
                    tprev = None
                    for q in range(3):
                        ck = pspool.tile([128, 2048], F32, tag="ps")
                        emit_chunk(ck, w_t, m_t, nt, q)
                        sk = sbpool.tile([128, 2048], F16, tag="sc")
                        nc.scalar.copy(out=sk[:], in_=ck[:])
                        if q == 0:
                            s0 = sk
                        elif q == 1:
                            t1 = tpool.tile([128, 2048], F16, tag="t1")
                            nc.vector.tensor_tensor(out=t1[:], in0=s0[:],
                                                    in1=sk[:], op=MIN)
                            tprev = t1
                        else:
                            t2 = tpool.tile([128, 2048], F16, tag="t2")
                            nc.vector.tensor_tensor(out=t2[:], in0=tprev[:],
                                                    in1=sk[:], op=MIN)
                            tprev = t2
                    # fold 2048 -> 1024 in 2x mode, then 1x reduce of 1024
                    u = tpool.tile([128, 1024], F16, tag="u")
                    nc.vector.tensor_tensor(out=u[:], in0=tprev[:, 0:1024],
                                            in1=tprev[:, 1024:2048], op=MIN)
                    nc.vector.tensor_reduce(out=strip_b[:, nt:nt + 1],
                                            in_=u[:],
                                            axis=mybir.AxisListType.X, op=MIN)
                # combine both strips, relu, sqrt with accumulation
                strip = stpool.tile([128, NT], F32, tag="strip")
                nc.vector.tensor_tensor(out=strip[:], in0=strip_a[:],
                                        in1=strip_b[:], op=MIN)
                relu_t = stpool.tile([128, NT], F32, tag="relu")
                nc.vector.tensor_scalar(out=relu_t[:], in0=strip[:],
                                        scalar1=0.0, scalar2=None,
                                        op0=mybir.AluOpType.max)
                sqrt_t = stpool.tile([128, NT], F32, tag="sqrt")
                persum = stpool.tile([128, 1], F32, tag="persum")
                nc.scalar.activation(out=sqrt_t[:], in_=relu_t[:],
                                     func=mybir.ActivationFunctionType.Sqrt,
                                     accum_out=persum[:])
                nc.sync.dma_start(out=out_d[p:p + 1, :], in_=persum[:])
    nc.compile()
    return nc


def _get_nc():
    global _NC_CACHE
    if _NC_CACHE is None:
        _NC_CACHE = _build_nc()
    return _NC_CACHE


def kernel(array1: np.ndarray, array2: np.ndarray) -> np.ndarray:
    array1 = np.asarray(array1, dtype=np.float32)
    array2 = np.asarray(array2, dtype=np.float32)
    assert array1.shape == (B, N, 3) and array2.shape == (B, N, 3)

    in_maps = []
    for c in range(B):
        wa, ma = _operands(array1[c])
        wb, mb = _operands(array2[c])
        in_maps.append({"wa": wa, "ma": ma, "wb": wb, "mb": mb})

    nc = _get_nc()
    res = run_bass_kernel_spmd(nc, in_maps, list(range(B))).results

    s1 = 0.0
    s2 = 0.0
    for c in range(B):
        o = res[c]["out"].astype(np.float64)
        s1 += o[0].sum()
        s2 += o[1].sum()
    val = 0.5 * (s1 / (B * N) + s2 / (B * N))
    return np.float32(val)



# revision 9
# speedup vs baseline: 7.7410x; 7.7410x over previous
"""Chamfer loss on 8 Trainium2 NeuronCores — windowed candidates + exact
tier-2 fallback.

Data-parallel over batch B=8: core c handles batch element c.

Host preprocessing (per batch element): sort both point sets by their
x-coordinate.  The x-gap lower-bounds the Euclidean distance, so a
query's nearest neighbour lies within x-rank window whose width scales
with its NN distance.  A cheap probe (distance to the +-128 rank
neighbours, O(N*256) host flops) yields a certified upper bound r_ub on
every point's NN distance, hence a certified candidate rank interval
[x - r_ub, x + r_ub].  Points whose interval fits their block's fixed
512-wide tier-1 window (>99% of points) are exactly solved by tier 1;
the few dozen others are exactly solved by a padded 128-query tier-2
full scan per direction.  The final result is exact up to fp16 rounding
of individual distances.

Device algorithm (per core):
  tier 1: for each of 64 query blocks (128 sorted a-points), ONE K=24
  bf16 matmul (fp32 coords split into 3 bf16 components each: 6
  cross-product rows per coordinate + 3 rows per squared norm keep
  ~1e-7 absolute accuracy at full PE rate) produces the [128, 512]
  fp32 distance tile in PSUM.  ScalarE converts it to fp16 in SBUF.
  VectorE row-min-reduces it with one tensor_tensor_scan (min,min,
  stride-0 broadcast output) -> a->b minima, and folds it into a
  persistent per-lane column-min accumulator with one 2x-mode
  tensor_tensor min -> b->a partial minima.  Each distance value costs
  one ScalarE touch + one DVE cycle.  The accumulator [128, 8192] is
  finished with PE transposes (fp16 -> fp16 PSUM) + strided
  tensor_reduce mins.
  tier 2: 128 gathered queries per direction scan all 8192 candidates
  (16 matmuls, chained scans).
Per-point minima ship to the host, which substitutes tier-2 values for
the flagged points and does relu/sqrt/mean in fp64.
"""

import numpy as np
import ml_dtypes

import concourse.bass as bass
import concourse.mybir as mybir
import concourse.tile as tile
from concourse import bacc
from concourse.bass_utils import run_bass_kernel_spmd

B = 8
N = 8192          # points per set
K = 24            # augmented contraction rows
NT = N // 128     # 64 blocks of 128 sorted points
W1 = 512          # tier-1 candidate window per query block
T2 = 128          # tier-2 queries per direction (padded)
T2C = 512         # tier-2 candidate chunk (PSUM-bank-limited matmul width)
TG = 8            # accumulator tiles per transpose group
KPROBE = 128      # host probe: +-KPROBE rank neighbours bound the NN dist
F32 = mybir.dt.float32
F16 = mybir.dt.float16
BF16 = mybir.dt.bfloat16
BF = ml_dtypes.bfloat16
BIG = 60000.0     # fp16-safe "infinity"

_NC_CACHE = None


def _split3(v32: np.ndarray):
    """fp32 -> (hi, mid, lo) bf16 triple with hi+mid+lo == v to ~2^-24 rel."""
    v1 = v32.astype(BF)
    r = v32 - v1.astype(np.float32)
    v2 = r.astype(BF)
    v3 = (r - v2.astype(np.float32)).astype(BF)
    return v1, v2, v3


def _w_side(pts: np.ndarray):
    """pts [n,3] fp32 -> w [24,n] bf16 stationary-side operand.

    Row pairing (per coordinate k, g = split3(-2*coord), h = split3(coord)):
      w rows: g1 g1 g2 g2 g1 g3   (m rows: h1 h2 h1 h2 h3 h1)
    so sum_r w[r]*m[r] = -2*coord_a*coord_b up to ~2^-26 terms.
    Rows 18-20: split3(||.||^2) against ones; rows 21-23: ones against the
    other side's split3(||.||^2).
    """
    s = (pts.astype(np.float64) ** 2).sum(axis=1).astype(np.float32)
    s1, s2, s3 = _split3(s)
    w = np.empty((K, pts.shape[0]), dtype=BF)
    for k in range(3):
        c = pts[:, k].astype(np.float32)
        g1, g2, g3 = _split3(-2.0 * c)
        r = 6 * k
        w[r + 0], w[r + 1], w[r + 2] = g1, g1, g2
        w[r + 3], w[r + 4], w[r + 5] = g2, g1, g3
    one = np.ones(pts.shape[0], dtype=BF)
    w[18], w[19], w[20] = s1, s2, s3
    w[21], w[22], w[23] = one, one, one
    return w


def _m_side(pts: np.ndarray):
    """pts [n,3] fp32 -> m [24,n] bf16 moving-side operand (see _w_side)."""
    s = (pts.astype(np.float64) ** 2).sum(axis=1).astype(np.float32)
    s1, s2, s3 = _split3(s)
    m = np.empty((K, pts.shape[0]), dtype=BF)
    for k in range(3):
        c = pts[:, k].astype(np.float32)
        h1, h2, h3 = _split3(c)
        r = 6 * k
        m[r + 0], m[r + 1], m[r + 2] = h1, h2, h1
        m[r + 3], m[r + 4], m[r + 5] = h2, h3, h1
    one = np.ones(pts.shape[0], dtype=BF)
    m[18], m[19], m[20] = one, one, one
    m[21], m[22], m[23] = s1, s2, s3
    return m


def _win_start(i: int) -> int:
    """Tier-1 window start (rank-centred on block i, clamped)."""
    return min(max(i * 128 + 64 - W1 // 2, 0), N - W1)


def _build_nc():
    nc = bacc.Bacc(None)
    wa_d = nc.declare_dram_parameter("wa", [K, N], BF16, isOutput=False)
    mb_d = nc.declare_dram_parameter("mb", [K, N], BF16, isOutput=False)
    ma_d = nc.declare_dram_parameter("ma", [K, N], BF16, isOutput=False)
    w2a_d = nc.declare_dram_parameter("w2a", [K, T2], BF16, isOutput=False)
    w2b_d = nc.declare_dram_parameter("w2b", [K, T2], BF16, isOutput=False)
    eye_d = nc.declare_dram_parameter("eye", [128, 128], F32, isOutput=False)
    sa_d = nc.declare_dram_parameter("sa", [128, NT], F32, isOutput=True)
    sb_d = nc.declare_dram_parameter("sb", [128, NT], F32, isOutput=True)
    t2_d = nc.declare_dram_parameter("t2", [2, 128], F32, isOutput=True)

    MIN = mybir.AluOpType.min
    NG = NT // TG  # transpose groups

    with tile.TileContext(nc) as tc:
        with (
            tc.tile_pool(name="const", bufs=1) as cpool,
            tc.tile_pool(name="psum", bufs=2, space="PSUM") as pspool,
            tc.tile_pool(name="psum2", bufs=2, space="PSUM") as ps2pool,
            tc.tile_pool(name="tpsum", bufs=2, space="PSUM") as tppool,
            tc.tile_pool(name="scopy", bufs=3) as sbpool,
            tc.tile_pool(name="scopy2", bufs=2) as sb2pool,
        ):
            wa_t = cpool.tile([K, N], BF16, tag="wa")
            mb_t = cpool.tile([K, N], BF16, tag="mb")
            ma_t = cpool.tile([K, N], BF16, tag="ma")
            w2a_t = cpool.tile([K, T2], BF16, tag="w2a")
            w2b_t = cpool.tile([K, T2], BF16, tag="w2b")
            eyef_t = cpool.tile([128, 128], F32, tag="eyef")
            eye_t = cpool.tile([128, 128], F16, tag="eye")
            for t, dram in ((wa_t, wa_d), (mb_t, mb_d), (ma_t, ma_d),
                            (w2a_t, w2a_d), (w2b_t, w2b_d), (eyef_t, eye_d)):
                nc.sync.dma_start(out=t[:], in_=dram[:])
            nc.scalar.copy(out=eye_t[:], in_=eyef_t[:])

            # persistent per-lane column-min accumulator (b->a partials)
            acc = cpool.tile([128, N], F16, tag="acc")
            nc.gpsimd.memset(acc[:], BIG)

            strip_a = cpool.tile([128, NT], F32, tag="stripa")
            strip_b = cpool.tile([128, NT], F32, tag="stripb")
            t2strip = cpool.tile([128, 2], F32, tag="t2strip")

            # ---- tier 1 ----
            for i in range(NT):
                s0 = _win_start(i)
                ck = pspool.tile([128, W1], F32, tag="ps")
                nc.tensor.matmul(
                    out=ck[:],
                    lhsT=wa_t[:, i * 128:(i + 1) * 128],
                    rhs=mb_t[:, s0:s0 + W1],
                    start=True, stop=True)
                sk = sbpool.tile([128, W1], F16, tag="sc")
                nc.scalar.copy(out=sk[:], in_=ck[:])
                # a->b row-min: one scan consumes both halves; final state
                # lands in the strip cell via a stride-0 broadcast output
                nc.vector.tensor_tensor_scan(
                    out=strip_a[:, i:i + 1].broadcast_to([128, W1 // 2]),
                    data0=sk[:, 0:W1 // 2],
                    data1=sk[:, W1 // 2:W1],
                    initial=BIG, op0=MIN, op1=MIN)
                # b->a per-lane column mins (2x mode)
                nc.vector.tensor_tensor(
                    out=acc[:, s0:s0 + W1], in0=sk[:],
                    in1=acc[:, s0:s0 + W1], op=MIN)

            # ---- tier 2: 128 gathered queries x all 8192, per direction ----
            for d, (w2_t, m_t) in enumerate(((w2a_t, mb_t), (w2b_t, ma_t))):
                cell = t2strip[:, d:d + 1]
                for q in range(N // (2 * T2C)):
                    ck = ps2pool.tile([128, 2 * T2C], F32, tag="ps2")
                    for h in range(2):
                        nc.tensor.matmul(
                            out=ck[:, h * T2C:(h + 1) * T2C],
                            lhsT=w2_t[:],
                            rhs=m_t[:, q * 2 * T2C + h * T2C:
                                    q * 2 * T2C + (h + 1) * T2C],
                            start=True, stop=True)
                    sk = sb2pool.tile([128, 2 * T2C], F16, tag="sc2")
                    nc.scalar.copy(out=sk[:], in_=ck[:])
                    nc.vector.tensor_tensor_scan(
                        out=cell.broadcast_to([128, T2C]),
                        data0=sk[:, 0:T2C],
                        data1=sk[:, T2C:2 * T2C],
                        initial=(BIG if q == 0 else cell),
                        op0=MIN, op1=MIN)

            # ---- finish b->a: transpose acc, reduce over original lanes ----
            for g in range(NG):
                tp = tppool.tile([128, TG * 128], F16, tag="tp")
                for j in range(TG):
                    t = g * TG + j
                    nc.tensor.transpose(
                        out=tp[:, j * 128:(j + 1) * 128],
                        in_=acc[:, t * 128:(t + 1) * 128],
                        identity=eye_t[:])
                nc.vector.tensor_reduce(
                    out=strip_b[:, g * TG:(g + 1) * TG],
                    in_=tp[:].rearrange("p (t x) -> p t x", t=TG),
                    axis=mybir.AxisListType.X, op=MIN)

            nc.sync.dma_start(out=sa_d[:], in_=strip_a[:])
            nc.sync.dma_start(out=sb_d[:], in_=strip_b[:])
            nc.sync.dma_start(out=t2_d[0:1, :], in_=t2strip[:, 0:1])
            nc.sync.dma_start(out=t2_d[1:2, :], in_=t2strip[:, 1:2])
    nc.compile()
    return nc


def _get_nc():
    global _NC_CACHE
    if _NC_CACHE is None:
        _NC_CACHE = _build_nc()
    return _NC_CACHE


def _probe_rub(q_s: np.ndarray, c_s: np.ndarray) -> np.ndarray:
    """Certified upper bound on each sorted query's NN distance: min dist
    to the +-KPROBE rank-neighbours in the sorted candidate set."""
    n, m = len(q_s), len(c_s)
    pos = np.searchsorted(c_s[:, 0], q_s[:, 0]).astype(np.int64)
    # gather a [n, 2*KPROBE] window of candidate indices (clamped)
    base = np.clip(pos - KPROBE, 0, m - 2 * KPROBE)
    idx = base[:, None] + np.arange(2 * KPROBE)[None, :]
    cand = c_s[idx]                                   # [n, 2K, 3]
    dd = ((q_s[:, None, :] - cand) ** 2).sum(-1).min(axis=1)
    return np.sqrt(dd) * (1 + 1e-6) + 1e-9


def _unsafe_sets(a_s: np.ndarray, b_s: np.ndarray):
    """Indices (in sorted order) of points whose certified candidate
    interval exceeds their tier-1 coverage."""
    rua = _probe_rub(a_s, b_s)
    rub = _probe_rub(b_s, a_s)
    alo = np.searchsorted(b_s[:, 0], a_s[:, 0] - rua)
    ahi = np.searchsorted(b_s[:, 0], a_s[:, 0] + rua)
    blo = np.searchsorted(a_s[:, 0], b_s[:, 0] - rub)
    bhi = np.searchsorted(a_s[:, 0], b_s[:, 0] + rub)
    starts = np.array([_win_start(i) for i in range(NT)])
    ws = starts[np.arange(N) // 128]
    unsafe_a = np.nonzero((alo < ws) | (ahi > ws + W1))[0]
    cov_lo = np.full(N, N, dtype=np.int64)
    cov_hi = np.zeros(N, dtype=np.int64)
    for i in range(NT):
        s = starts[i]
        cov_lo[s:s + W1] = np.minimum(cov_lo[s:s + W1], i * 128)
        cov_hi[s:s + W1] = np.maximum(cov_hi[s:s + W1], (i + 1) * 128)
    unsafe_b = np.nonzero((blo < cov_lo) | (bhi > cov_hi))[0]
    return unsafe_a, unsafe_b


def _in_maps(array1: np.ndarray, array2: np.ndarray):
    eye = np.eye(128, dtype=np.float32)
    in_maps = []
    meta = []
    for c in range(B):
        a_s = array1[c][np.argsort(array1[c][:, 0], kind="stable")]
        b_s = array2[c][np.argsort(array2[c][:, 0], kind="stable")]
        ua, ub = _unsafe_sets(a_s, b_s)
        assert len(ua) <= T2 and len(ub) <= T2, (len(ua), len(ub))
        wa = _w_side(a_s)
        wb = _w_side(b_s)
        w2a = np.zeros((K, T2), dtype=BF)
        w2b = np.zeros((K, T2), dtype=BF)
        w2a[:, :len(ua)] = wa[:, ua]
        w2b[:, :len(ub)] = wb[:, ub]
        in_maps.append({"wa": wa, "mb": _m_side(b_s), "ma": _m_side(a_s),
                        "w2a": w2a, "w2b": w2b, "eye": eye})
        meta.append((ua, ub))
    return in_maps, meta


def kernel(array1: np.ndarray, array2: np.ndarray) -> np.ndarray:
    array1 = np.asarray(array1, dtype=np.float32)
    array2 = np.asarray(array2, dtype=np.float32)
    assert array1.shape == (B, N, 3) and array2.shape == (B, N, 3)

    in_maps, meta = _in_maps(array1, array2)
    nc = _get_nc()
    res = run_bass_kernel_spmd(nc, in_maps, list(range(B))).results

    s1 = 0.0
    s2 = 0.0
    for c in range(B):
        ua, ub = meta[c]
        mina = res[c]["sa"].astype(np.float64).T.reshape(-1)  # [N] by rank
        minb = res[c]["sb"].astype(np.float64).T.reshape(-1)
        t2v = res[c]["t2"].astype(np.float64)
        mina[ua] = t2v[0, :len(ua)]
        minb[ub] = t2v[1, :len(ub)]
        s1 += np.sqrt(np.maximum(mina, 0.0)).sum()
        s2 += np.sqrt(np.maximum(minb, 0.0)).sum()
    val = 0.5 * (s1 / (B * N) + s2 / (B * N))
    return np.float32(val)
